# revision 4
# baseline (speedup 1.0000x reference)
"""MiniBatchSemiNMF encode kernel for Trainium2 (8 NeuronCores, Bass/Tile).

Data-parallel over the batch (1024 rows/core), transposed layout (k on
partitions, batch on free). The 20-iteration multiplicative-update loop runs
its two KxK matvec batches in fp8e4 DoubleRow (2x PE rate), made accurate
enough (validated in a bit-faithful numpy sim against the fp32 reference) by
phase-dithered quantization: two prequantized copies of ddt_pos/neg (scaled
1/gamma_ph) alternate across iterations, paired with a gamma_ph-scaled fp8
cast of z, so the frozen rounding bias (which the slow "ridge" modes amplify
by up to ~n_iters x) alternates sign and largely cancels.

Engine balance (the point of this revision): per iteration each k-tile runs
only 6 PE matmuls (4 fp8-DR math + 2 f16 folds) instead of 8 --
  - the denominator's constant atd_neg term and the exact diagonal both ride
    ONE f16 matmul diagw @ m2, where m2 = zm + snd and snd[ph] =
    g_ph*ALPHA*atd_neg_floor/diag is a host-precomputed f16 constant; m2 is
    maintained by the otherwise-idle GpSimd engine (one f16 add per tile);
  - the numerator's constant atd_pos term rides an f16 identity matmul
    (I @ spf), replacing the fp8 hi/lo DoubleRow fold (also more accurate:
    f16 vs paired-fp8 is ~8x tighter).
Elementwise work runs at free-dim 1024 (full per-core batch, PSUM tiles
spanning two banks) to halve per-op fixed overheads: DVE does the warm
rsqrt-Newton q-state (custom op, q' = q*max(1.5 - 0.5*den*q^2, 0.25)) plus
the two f16 muls (f = h*q', zm' = zm*f); Act does sqrt (h, PSUM source) and
the fp8 cast of the z-master (same activation table set, no reloads).
atd-derived constants and the f16 z0 master are precomputed exactly on the
host in make_in_maps (like the baseline's host-side ddt/inv cache terms).
"""

import sys

for _p in ("/opt/trn_rl_repo",):
    if _p not in sys.path:
        sys.path.insert(0, _p)

import numpy as np
import ml_dtypes

import concourse.bacc as bacc
import concourse.tile as tile
from concourse import mybir
from concourse.bass_utils import run_bass_kernel_spmd

from concourse import dve_ops
from concourse.dve_spec import C0, C1, C2, Spec, Src0, Src1, lower, maxx
from concourse.dve_uop import DveOpSpec

# --- custom DVE op: warm rsqrt Newton step (single PSUM input) -------------
# out = Src1 * max(C0 - C1*(Src0*Src1^2), C2); C0=1.5 C1=0.5 C2=clamp.
_RSQ_SPEC = Spec(
    body=Src1 * maxx(C0 - C1 * (Src0 * (Src1 * Src1)), C2),
    reference=lambda in0, in1, c0, c1, c2: (
        in1 * np.maximum(c0 - c1 * (in0 * (in1.astype(np.float32) ** 2)), c2)
    ),
)
_RSQ_NAME = "RSQRT_WARM_NR_ANT"
_RSQ_OP = None


def _register_rsqrt_nr():
    global _RSQ_OP
    if _RSQ_OP is not None:
        return _RSQ_OP
    for op in dve_ops.OPS:
        if op.name == _RSQ_NAME:
            _RSQ_OP = op
            return op
    row = dve_ops._CUSTOM_DVE_ROW_BASE + len(dve_ops.OPS)
    assert row < 0x20, "custom-DVE opcode row field is 5 bits"
    shas = {}
    for ver in ("v3", "v4"):
        s = DveOpSpec(
            name=_RSQ_NAME, opcode=row, uops=lower(_RSQ_SPEC, ver=ver), rd1_en=True
        )
        shas[ver] = s.sha(ver)
    op = dve_ops.DveOp(_RSQ_NAME, _RSQ_SPEC, subdim=False, uops_sha=shas)
    dve_ops.OPS.append(op)
    dve_ops._SUB_OPCODE_FOR_NAME[_RSQ_NAME] = row
    dve_ops.CUSTOM_DVE_SPECS[_RSQ_NAME] = _RSQ_SPEC
    _RSQ_OP = op
    return op


E4NP = ml_dtypes.float8_e4m3
F32 = mybir.dt.float32
F16 = mybir.dt.float16
F8 = mybir.dt.float8e4

EPS = 1e-8
N_CORES = 8
B, DM, K = 8192, 1024, 512
R = B // N_CORES  # 1024 rows per core
RC = 512  # matmul moving-chunk width (one PSUM bank of f32)
NRC = R // RC  # 2
NK = K // 128  # 4 output k-tiles
NKP = K // 256  # 2 DoubleRow contraction pair-tiles

ALPHA = 256.0  # common PSUM scale
GAMMA = 0.031  # dither amplitude
DEN_FLOOR = 1e-4
NR_CLAMP = 0.25

_BUILD_CACHE: dict[int, object] = {}


def _build(n_iters: int):
    rsq_op = _register_rsqrt_nr()
    nc = bacc.Bacc("TRN2", target_bir_lowering=False, debug=False, num_devices=N_CORES)

    # --- dram inputs (host precomputes atd-derived constants and z0) ---
    spf_d = nc.dram_tensor("spf", [K, R], F16, kind="ExternalInput").ap()
    snd_d = [
        nc.dram_tensor(f"snd{p}", [K, R], F16, kind="ExternalInput").ap()
        for p in range(2)
    ]
    zm0_d = nc.dram_tensor("zm0", [K, R], F16, kind="ExternalInput").ap()
    dpos_d = [
        nc.dram_tensor(f"dpos{p}", [NKP * 128, 2, K], F8, kind="ExternalInput").ap()
        for p in range(2)
    ]
    dneg_d = [
        nc.dram_tensor(f"dneg{p}", [NKP * 128, 2, K], F8, kind="ExternalInput").ap()
        for p in range(2)
    ]
    diagw_d = [
        nc.dram_tensor(f"diagw{p}", [128, K], F16, kind="ExternalInput").ap()
        for p in range(2)
    ]
    i16_d = nc.dram_tensor("i16", [128, 128], F16, kind="ExternalInput").ap()
    out_d = nc.dram_tensor("zT", [K, R], F32, kind="ExternalOutput").ap()

    Sqrt = mybir.ActivationFunctionType.Sqrt
    Copy = mybir.ActivationFunctionType.Copy
    DR = mybir.MatmulPerfMode.DoubleRow

    gammas = [1.0 + GAMMA, 1.0 - GAMMA]

    with tile.TileContext(nc) as tc:
        with (
            tc.tile_pool(name="weights", bufs=1) as wp,
            tc.tile_pool(name="zm", bufs=2 * NK) as zmp,
            tc.tile_pool(name="zq", bufs=2 * NKP) as zqp,
            tc.tile_pool(name="qs", bufs=2 * NK) as qsp,
            tc.tile_pool(name="m2", bufs=2 * NK) as m2p,
            tc.tile_pool(name="tmp", bufs=4) as tmpp,
            tc.tile_pool(name="psum", bufs=2, space="PSUM") as psp,
        ):
            qeng = [nc.sync, nc.gpsimd, nc.scalar, nc.gpsimd]
            # --- persistent weights/stationaries ---
            i16 = wp.tile([128, 128], F16, name="i16_sb", tag="i16")
            nc.sync.dma_start(i16[:], i16_d[:])
            diagw = []
            for p in range(2):
                t = wp.tile([128, K], F16, name=f"diagw_sb{p}", tag=f"diagw{p}")
                nc.sync.dma_start(t[:], diagw_d[p][:])
                diagw.append(t)
            dpos_sb = [[None] * NKP for _ in range(2)]
            dneg_sb = [[None] * NKP for _ in range(2)]
            for p in range(2):
                for kq in range(NKP):
                    rows = slice(kq * 128, (kq + 1) * 128)
                    t = wp.tile([128, 2, K], F8, name=f"dpos{p}_{kq}", tag=f"dpos{p}_{kq}")
                    qeng[(p * 2 + kq) % 4].dma_start(t[:], dpos_d[p][rows, :, :])
                    dpos_sb[p][kq] = t
                    t = wp.tile([128, 2, K], F8, name=f"dneg{p}_{kq}", tag=f"dneg{p}_{kq}")
                    qeng[(p * 2 + kq + 1) % 4].dma_start(t[:], dneg_d[p][rows, :, :])
                    dneg_sb[p][kq] = t

            # --- per-core batch constants: spf, snd[ph], zm0 ---
            spf_sb = []
            snd_sb = [[None] * NK for _ in range(2)]
            for kp in range(NK):
                rows = slice(kp * 128, (kp + 1) * 128)
                t = wp.tile([128, R], F16, name=f"spf_{kp}", tag=f"spf{kp}")
                qeng[kp % 4].dma_start(t[:], spf_d[rows, :])
                spf_sb.append(t)
                for p in range(2):
                    t = wp.tile([128, R], F16, name=f"snd{p}_{kp}", tag=f"snd{p}_{kp}")
                    qeng[(kp + p + 1) % 4].dma_start(t[:], snd_d[p][rows, :])
                    snd_sb[p][kp] = t

            zm_sb = [[None] * NK for _ in range(2)]
            q_sb = [[None] * NK for _ in range(2)]
            m2_sb = [[None] * NK for _ in range(2)]
            zq_pack = [[None] * NKP for _ in range(2)]
            for st in range(2):
                for kp in range(NK):
                    zm_sb[st][kp] = zmp.tile([128, R], F16, name=f"zm{st}_{kp}", tag="zm")
                    q_sb[st][kp] = qsp.tile([128, R], F16, name=f"q{st}_{kp}", tag="qs")
                    m2_sb[st][kp] = m2p.tile([128, R], F16, name=f"m2{st}_{kp}", tag="m2")
                for kq in range(NKP):
                    zq_pack[st][kq] = zqp.tile(
                        [128, 2, R], F8, name=f"zq{st}_{kq}", tag="zq"
                    )
            for kp in range(NK):
                rows = slice(kp * 128, (kp + 1) * 128)
                qeng[(kp + 2) % 4].dma_start(zm_sb[0][kp][:], zm0_d[rows, :])
            for kp in range(NK):
                kq, sub = divmod(kp, 2)
                nc.scalar.activation(
                    zq_pack[0][kq][:, sub, :], zm_sb[0][kp][:], Copy
                )
                nc.gpsimd.tensor_add(
                    m2_sb[0][kp][:], zm_sb[0][kp][:], snd_sb[0][kp][:]
                )

            # --- iteration loop (rc-chunked so iteration t+1's matmuls can
            # start as soon as chunk rc=0's casts/m2 land: cross-iteration
            # pipelining) ---
            for t_it in range(n_iters):
                cur, nxt = t_it % 2, (t_it + 1) % 2
                ph = t_it % 2
                g_ratio = gammas[(t_it + 1) % 2] / gammas[t_it % 2]
                for rc in range(NRC):
                    half = slice(rc * RC, (rc + 1) * RC)
                    for kp in range(NK):
                        kcols = slice(kp * 128, (kp + 1) * 128)
                        pd = psp.tile([128, RC], F32, name=f"pd{t_it}_{rc}_{kp}", tag="pd")
                        for kq in range(NKP):
                            nc.tensor.matmul(
                                pd[:], dpos_sb[ph][kq][:, :, kcols],
                                zq_pack[cur][kq][:, :, half],
                                start=(kq == 0), stop=False, perf_mode=DR,
                            )
                        nc.tensor.matmul(
                            pd[:], diagw[ph][:, kcols], m2_sb[cur][kp][:, half],
                            start=False, stop=True,
                        )
                        pn = psp.tile([128, RC], F32, name=f"pn{t_it}_{rc}_{kp}", tag="pn")
                        for kq in range(NKP):
                            nc.tensor.matmul(
                                pn[:], dneg_sb[ph][kq][:, :, kcols],
                                zq_pack[cur][kq][:, :, half],
                                start=(kq == 0), stop=False, perf_mode=DR,
                            )
                        nc.tensor.matmul(
                            pn[:], i16[:], spf_sb[kp][:, half],
                            start=False, stop=True,
                        )
                        # q state: warm rsqrt-NR (init via approx recip + sqrt)
                        q_new = q_sb[nxt][kp]
                        if t_it == 0:
                            r0 = tmpp.tile([128, RC], F32, name=f"r0_{rc}_{kp}", tag="r0", bufs=2)
                            nc.vector.reciprocal_approx_fast(r0[:], pd[:])
                            nc.scalar.activation(q_new[:, half], r0[:], Sqrt)
                        else:
                            nc.vector._custom_dve(
                                rsq_op, out=q_new[:, half], in0=pd[:],
                                in1=q_sb[cur][kp][:, half],
                                s0=1.5, s1=0.5, imm2=NR_CLAMP,
                            )
                        h = tmpp.tile([128, RC], F16, name=f"h{t_it}_{rc}_{kp}", tag="h")
                        nc.scalar.activation(h[:], pn[:], Sqrt, scale=g_ratio * g_ratio)
                        f = tmpp.tile([128, RC], F16, name=f"f{t_it}_{rc}_{kp}", tag="f")
                        nc.vector.tensor_mul(f[:], h[:], q_new[:, half])
                        zm_new = zm_sb[nxt][kp]
                        nc.vector.tensor_mul(
                            zm_new[:, half], zm_sb[cur][kp][:, half], f[:]
                        )
                        if t_it < n_iters - 1:
                            kq, sub = divmod(kp, 2)
                            nc.scalar.activation(
                                zq_pack[nxt][kq][:, sub, half], zm_new[:, half], Copy
                            )
                            nc.gpsimd.tensor_add(
                                m2_sb[nxt][kp][:, half], zm_new[:, half],
                                snd_sb[nxt][kp][:, half],
                            )

            # --- output: z = zm / (alpha*gamma_fin), copies split Act/DVE ---
            fin = n_iters % 2
            oscale = 1.0 / (ALPHA * gammas[fin])
            for kp in range(NK):
                zo = tmpp.tile([128, R], F32, name=f"zo{kp}", tag="zo", bufs=2)
                if kp % 2 == 0:
                    nc.scalar.activation(zo[:], zm_sb[fin][kp][:], Copy, scale=oscale)
                else:
                    nc.vector.tensor_scalar_mul(zo[:], zm_sb[fin][kp][:], oscale)
                qeng[kp % 4].dma_start(out_d[kp * 128 : (kp + 1) * 128, :], zo[:])

    nc.compile()
    return nc


def _get_program(n_iters: int):
    if n_iters not in _BUILD_CACHE:
        _BUILD_CACHE[n_iters] = _build(n_iters)
    return _BUILD_CACHE[n_iters]


def _q8(x):
    return np.clip(x, -240, 240).astype(E4NP)


def make_in_maps(acts: np.ndarray, D: np.ndarray):
    """Host-side prep: fp8 dithered ddt copies, f16 atd constants, z0."""
    acts = np.ascontiguousarray(acts, dtype=np.float32)
    D = np.ascontiguousarray(D, dtype=np.float32)
    ddt = (D.astype(np.float64) @ D.T.astype(np.float64)).astype(np.float32)
    ddt_pos = ((np.abs(ddt) + ddt) * 0.5).astype(np.float32)
    ddt_neg = ((np.abs(ddt) - ddt) * 0.5).astype(np.float32)
    diag = np.diag(ddt_pos).copy()
    dpos_nd = ddt_pos - np.diag(diag)
    eye_k = np.eye(K, dtype=np.float64)
    inv = np.linalg.solve(ddt.astype(np.float64) + EPS * eye_k, eye_k)

    gammas = [1.0 + GAMMA, 1.0 - GAMMA]

    def pack_dr(M):  # [K, K] -> [NKP*128, 2, K]
        out = np.empty((NKP * 128, 2, K), dtype=E4NP)
        for kq in range(NKP):
            for i in range(2):
                rows = M[kq * 256 + i * 128 : kq * 256 + (i + 1) * 128, :]
                out[kq * 128 : (kq + 1) * 128, i, :] = _q8(rows)
        return out

    dpos_p = [pack_dr(dpos_nd / g) for g in gammas]
    dneg_p = [pack_dr(ddt_neg / g) for g in gammas]

    diagw_p = []
    for g in gammas:
        dw = np.zeros((128, K), dtype=np.float16)
        for kp in range(NK):
            blk = diag[kp * 128 : (kp + 1) * 128] / g
            dw[:, kp * 128 : (kp + 1) * 128] = np.diag(blk.astype(np.float16))
        diagw_p.append(dw)

    i16 = np.eye(128, dtype=np.float16)

    # host-side atd-derived constants and z0 (exact fp32/fp64)
    atdT = (D.astype(np.float64) @ acts.T.astype(np.float64)).astype(np.float32)  # [K, B]
    spf = (np.maximum(atdT, 0.0) * np.float32(ALPHA)).astype(np.float16)
    atd_negf = np.maximum(-atdT, DEN_FLOOR)
    snd_p = [
        (np.float32(g * ALPHA) * atd_negf / diag[:, None]).astype(np.float16)
        for g in gammas
    ]
    z0T = np.maximum(atdT.astype(np.float64).T @ inv, EPS).T.astype(np.float32)  # [K, B]
    zm0 = (z0T * np.float32(ALPHA * (1.0 + GAMMA))).astype(np.float16)
    in_maps = []
    for c in range(N_CORES):
        cols = slice(c * R, (c + 1) * R)
        in_maps.append(
            {
                "spf": np.ascontiguousarray(spf[:, cols]),
                "snd0": np.ascontiguousarray(snd_p[0][:, cols]),
                "snd1": np.ascontiguousarray(snd_p[1][:, cols]),
                "zm0": np.ascontiguousarray(zm0[:, cols]),
                "dpos0": dpos_p[0],
                "dpos1": dpos_p[1],
                "dneg0": dneg_p[0],
                "dneg1": dneg_p[1],
                "diagw0": diagw_p[0],
                "diagw1": diagw_p[1],
                "i16": i16,
            }
        )
    return in_maps


def kernel(acts: np.ndarray, D: np.ndarray, n_iters) -> np.ndarray:
    n_iters = int(n_iters)
    nc = _get_program(n_iters)
    in_maps = make_in_maps(acts, D)
    z = None
    last_exc = None
    for attempt in range(3):
        try:
            res = run_bass_kernel_spmd(nc, in_maps, core_ids=list(range(N_CORES)))
        except Exception as exc:  # noqa: BLE001 - device flake, retried
            last_exc = exc
            import time

            time.sleep(2.0 * (attempt + 1))
            continue
        z = np.empty((B, K), dtype=np.float32)
        for c in range(N_CORES):
            z[c * R : (c + 1) * R, :] = res.results[c]["zT"].T
        if np.isfinite(z).all():
            return z
    if z is None:
        raise last_exc
    return z
